# revision 25
# baseline (speedup 1.0000x reference)
"""Trainium2 Bass kernel for Baichuan attention (B=2, S=1024, HID=4096, NH=32).

Sharding: tensor-parallel over heads (4 heads/core on 8 cores) for
QKV projection + rotary + causal attention; an AllToAll then redistributes
the attention output so every core holds all 4096 features for its own
256-token slice and computes those rows of the final o_proj output with the
full o_proj weight. Host-side gather is a pure concatenation.

On-chip layout is feature-major [feature, token]. QKV/score matmuls run in
float32r (TF32-class mantissa); the softmax weights, V, attention output and
o_proj weight use fp16, whose 10-bit mantissa matches float32r's, halving
their DMA/SBUF footprint at no accuracy cost.
"""
import numpy as np

import concourse.bass as bass
import concourse.mybir as mybir
import concourse.bacc as bacc
import concourse.tile as tile

NCORES = 8
B, S, HID, NH, HD = 2, 1024, 4096, 32, 128
HPC = NH // NCORES          # heads per core = 4
TQ = B * S                  # 2048 tokens
TSL = TQ // NCORES          # 256-token output slice per core
JC = HPC * HD               # 512 features per core per q/k/v
THETA = 10000.0

F32 = mybir.dt.float32
F32R = mybir.dt.float32r
F16 = mybir.dt.float16
AF = mybir.ActivationFunctionType
SCALE = float(HD) ** -0.5


def build_program():
    nc = bacc.Bacc("TRN2", target_bir_lowering=False, debug=False,
                   num_devices=NCORES)
    xT = nc.dram_tensor("xT", [HID, TQ], F16, kind="ExternalInput").ap()
    wT = nc.dram_tensor("wT", [HID, 3 * JC], F16, kind="ExternalInput").ap()
    opT = nc.dram_tensor("opT", [HID, HID], F16, kind="ExternalInput").ap()
    cosT = nc.dram_tensor("cosT", [128, TQ], F32R, kind="ExternalInput").ap()
    sinT = nc.dram_tensor("sinT", [128, TQ], F32R, kind="ExternalInput").ap()
    masks = nc.dram_tensor("masks", [128, 4 * 512], F16,
                           kind="ExternalInput").ap()
    onesI = nc.dram_tensor("onesI", [128, 8], F16, kind="ExternalInput").ap()
    out = nc.dram_tensor("out", [TSL, HID], F32, kind="ExternalOutput").ap()

    with tile.TileContext(nc) as tc:
        with tc.tile_pool(name="const", bufs=1) as cp, \
             tc.tile_pool(name="dramp", bufs=1, space="DRAM") as dramp:
            cos_sb = cp.tile([128, TQ], F32R)
            sin_sb = cp.tile([128, TQ], F32R)
            mask_sb = cp.tile([128, 4 * 512], F16)
            ones_sb = cp.tile([128, 8], F16)
            bias_sb = cp.tile([128, 1], F32)
            nc.vector.memset(bias_sb[:], -4.0)
            nc.scalar.dma_start(cos_sb[:], cosT)
            nc.scalar.dma_start(sin_sb[:], sinT)
            nc.scalar.dma_start(mask_sb[:], masks)
            nc.scalar.dma_start(ones_sb[:], onesI)

            # One AllToAll per batch: batch 0's exchange hides entirely
            # under batch 1's QKV. Each core ends up owning 128 tokens of
            # each batch (s in [c*128, (c+1)*128)).
            a2a_in = [dramp.tile([NCORES, JC, S // NCORES], F16,
                                 name=f"a2a_in{b}") for b in range(B)]
            a2a_out = [dramp.tile([NCORES, JC, S // NCORES], F16,
                                  name=f"a2a_out{b}") for b in range(B)]
            qT_dram = dramp.tile([B, JC, S], F16)

            # avall/opstr opened early so o_proj weight streaming can be
            # prefetched during attention; xslab pool shares per-d tags so
            # batch 1's activation DMAs overlap batch 0's attention.
            with tc.tile_pool(name="avall", bufs=1) as avp, \
                 tc.tile_pool(name="opstr", bufs=13) as opp, \
                 tc.tile_pool(name="psum", bufs=1, space="PSUM") as pspool, \
                 tc.tile_pool(name="xslab", bufs=1) as xp:
                for b in range(B):
                    with tc.tile_pool(name=f"qkv{b}", bufs=1) as qkvp:
                        kT = [qkvp.tile([128, S], F16, name=f"kT{b}_{h}")
                              for h in range(HPC)]
                        vv = [qkvp.tile([128, JC], F16, name=f"v{b}_{t}")
                              for t in range(8)]
                        _qkv_phase(nc, tc, b, xp, pspool, xT, wT, cos_sb,
                                   sin_sb, kT, vv, qT_dram)
                        _attn_phase(nc, tc, b, pspool, kT, vv, qT_dram,
                                    mask_sb, ones_sb, bias_sb, a2a_in[b])
                    nc.gpsimd.collective_compute(
                        "AllToAll", mybir.AluOpType.bypass,
                        replica_groups=[list(range(NCORES))],
                        ins=[a2a_in[b][:]], outs=[a2a_out[b][:]])

                _oproj_phase(nc, tc, pspool, a2a_out, opT, out, avp, opp)
    nc.compile()
    return nc


def _qkv_phase(nc, tc, b, xp, pspool, xT, wT, cos_sb, sin_sb, kT, vv,
               qT_dram):
    """QKV projection + RoPE for batch b.

    Q/K come out feature-major ([dh, t]; Q spilled to DRAM, K kept in SBUF),
    V token-major ([t, jv]) to serve directly as the AV stationary operand.
    """
    ND = HID // 128  # 32 contraction tiles
    with tc.tile_pool(name=f"wstr{b}", bufs=12) as wp, \
         tc.tile_pool(name=f"rope{b}", bufs=2) as rp, \
         tc.tile_pool(name=f"qev{b}", bufs=2) as qevp:
        xs = []

        # --- Q (jq=0) and K (jq=1), feature-major ---
        for jq in range(2):
            ps = [pspool.tile([128, 512], F32, name=f"ps{b}_{jq}_{i}",
                              tag=f"bk{i}") for i in range(8)]
            for d in range(ND):
                if jq == 0:
                    # just-in-time activation loads: x tile d arrives right
                    # before its first use instead of in one blocking burst
                    xt = xp.tile([128, S], F16, name=f"x{b}_{d}", tag=f"x{d}")
                    nc.sync.dma_start(xt[:], xT[d * 128:(d + 1) * 128,
                                                b * S:(b + 1) * S])
                    xs.append(xt)
                wt = wp.tile([128, 512], F16, tag="wt")
                nc.sync.dma_start(
                    wt[:], wT[d * 128:(d + 1) * 128, jq * 512:(jq + 1) * 512])
                for j in range(4):
                    for ts in range(2):
                        nc.tensor.matmul(
                            ps[j * 2 + ts][:],
                            wt[:, j * 128:(j + 1) * 128],
                            xs[d][:, ts * 512:(ts + 1) * 512],
                            start=(d == 0), stop=(d == ND - 1))
            # Evict all 8 accumulator banks first (alternating engines) so
            # the next pass's matmuls reclaim PSUM immediately; then do the
            # rotary math from SBUF.
            raws = []
            for j in range(4):
                for ts in range(2):
                    raw = rp.tile([128, 512], F16, tag=f"raw{j * 2 + ts}",
                                  bufs=1)
                    if (j + ts) % 2 == 0:
                        nc.scalar.copy(raw[:], ps[j * 2 + ts][:])
                    else:
                        nc.vector.tensor_copy(raw[:], ps[j * 2 + ts][:])
                    raws.append(raw)
            for j in range(4):
                for ts in range(2):
                    raw = raws[j * 2 + ts]
                    tq0 = b * S + ts * 512
                    csl = cos_sb[:, tq0:tq0 + 512]
                    ssl = sin_sb[:, tq0:tq0 + 512]
                    if jq == 1:
                        dest = kT[j][:, ts * 512:(ts + 1) * 512]
                    else:
                        qe = qevp.tile([128, 512], F16, tag="qe")
                        dest = qe[:]
                    sw = rp.tile([128, 512], F16, tag="sw")
                    for qd in range(4):
                        nc.vector.tensor_copy(
                            sw[qd * 32:(qd + 1) * 32, :],
                            raw[(qd * 32 + 64) % 128:
                                (qd * 32 + 64) % 128 + 32, :])
                    nc.vector.tensor_mul(dest, raw[:], csl)
                    nc.vector.tensor_mul(sw[:], sw[:], ssl)
                    nc.vector.tensor_add(dest, dest, sw[:])
                    if jq == 0:
                        nc.scalar.dma_start(
                            qT_dram[b, j * 128:(j + 1) * 128,
                                    ts * 512:(ts + 1) * 512], dest)

        # --- V (jq=2), token-major: psum[t-block] = x_tile.T @ w_v ---
        psv = [pspool.tile([128, 512], F32, name=f"psv{b}_{i}", tag=f"bk{i}")
               for i in range(8)]
        for d in range(ND):
            wt = wp.tile([128, 512], F16, tag="wt")
            nc.sync.dma_start(
                wt[:], wT[d * 128:(d + 1) * 128, 1024:1536])
            for t8 in range(8):
                nc.tensor.matmul(
                    psv[t8][:],
                    xs[d][:, t8 * 128:(t8 + 1) * 128],
                    wt[:],
                    start=(d == 0), stop=(d == ND - 1))
        for t8 in range(8):
            if t8 % 2 == 0:
                nc.scalar.copy(vv[t8][:], psv[t8][:])
            else:
                nc.vector.tensor_copy(vv[t8][:], psv[t8][:])


def _attn_phase(nc, tc, b, pspool, kT, vv, qT_dram, mask_sb, ones_sb,
                bias_sb, a2a_in):
    """Causal attention for batch b: softmax(Q K^T / sqrt(d)) V, 4 heads.

    Works on S^T = K Q^T tiles [k:128, q:512] so the contraction dim (dh,
    then k) always sits on partitions; softmax denominator via a ones-column
    matmul; no max-subtraction (scores are O(10), exp cannot overflow, and
    fp16 underflow of ~e^-18 tail weights is negligible). The score matmul
    runs two k-blocks ahead of the AV/denominator matmuls so the PE never
    waits on the exp/mask latency.
    """
    with tc.tile_pool(name=f"at{b}", bufs=1) as ap:
        cnt = [0, 0]
        for h in range(HPC):
            for qt in range(2):
                qtile = ap.tile([128, 512], F16, tag="qs", bufs=3)
                nc.scalar.dma_start(
                    qtile[:], qT_dram[b, h * 128:(h + 1) * 128,
                                      qt * 512:(qt + 1) * 512])
                psav = pspool.tile([128, 512], F32,
                                   name=f"psav{b}_{h}_{qt}",
                                   tag=f"bk{5 + cnt[1] % 2}")
                psds = pspool.tile([1, 512], F32, name=f"psds{b}_{h}_{qt}",
                                   tag="bk7")
                cnt[1] += 1
                nkb = 4 * (qt + 1)

                def score_tile(kb):
                    pss = pspool.tile([128, 512], F32,
                                      name=f"pss{b}_{h}_{qt}_{kb}",
                                      tag=f"bk{cnt[0] % 5}")
                    cnt[0] += 1
                    nc.tensor.matmul(
                        pss[:], kT[h][:, kb * 128:(kb + 1) * 128], qtile[:],
                        start=True, stop=True)
                    es = ap.tile([128, 512], F16, tag="es", bufs=6)
                    nc.scalar.activation(es[:], pss[:], AF.Exp, scale=SCALE,
                                         bias=bias_sb[:])
                    dd = kb - 4 * qt
                    if 0 <= dd < 4:
                        nc.vector.tensor_mul(
                            es[:], es[:], mask_sb[:, dd * 512:(dd + 1) * 512])
                    return es

                es_q = [score_tile(k) for k in range(min(4, nkb))]
                for kb in range(nkb):
                    if kb + 4 < nkb:
                        es_q.append(score_tile(kb + 4))
                    es = es_q.pop(0)
                    nc.tensor.matmul(
                        psav[:], vv[kb][:, h * 128:(h + 1) * 128], es[:],
                        start=(kb == 0), stop=(kb == nkb - 1))
                    nc.tensor.matmul(
                        psds[:], ones_sb[:, 0:1], es[:],
                        start=(kb == 0), stop=(kb == nkb - 1))
                recip = ap.tile([1, 512], F32, tag="recip", bufs=2)
                nc.vector.reciprocal_approx_fast(recip[:], psds[:])
                rbc = ap.tile([128, 512], F32, tag="rbc", bufs=2)
                nc.gpsimd.partition_broadcast(rbc[:], recip[:])
                avt = ap.tile([128, 512], F16, tag="avt", bufs=4)
                nc.vector.tensor_mul(avt[:], psav[:], rbc[:])
                for qr in range(4):
                    peer = qt * 4 + qr
                    nc.sync.dma_start(
                        a2a_in[peer, h * 128:(h + 1) * 128, :],
                        avt[:, qr * 128:(qr + 1) * 128])


def _oproj_phase(nc, tc, pspool, a2a_out, opT, out, avp, opp):
    """out rows = [batch0 tokens c*128..+128, batch1 same range] @ o_proj.T."""
    NJ = HID // 128  # 32
    with tc.tile_pool(name="oev", bufs=3) as oevp:
        sl = S // NCORES
        avc = []
        for bb in range(B):
            flat = a2a_out[bb].rearrange("a b c -> (a b) c")
            t = avp.tile([128, NJ * sl], F16, name=f"avc{bb}")
            src = flat.rearrange("(a p) t -> p a t", p=128)
            dst = t[:].rearrange("p (a t) -> p a t", a=NJ)
            for ch in range(8):
                nc.scalar.dma_start(dst[:, ch * 4:(ch + 1) * 4, :],
                                    src[:, ch * 4:(ch + 1) * 4, :])
            avc.append(t)
        # batch-1 (tb=1) matmuls run W iterations behind batch-0's so the
        # final AllToAll's ~30us latency hides under batch-0-only matmuls
        W = 12
        for half in range(2):
            ps = [pspool.tile([128, 512], F32, name=f"pso{half}_{i}",
                              tag=f"bk{i}") for i in range(8)]
            opts = {}
            for i in range(NJ + W):
                if i < NJ:
                    opt = opp.tile([128, 2048], F16, tag="op")
                    nc.scalar.dma_start(
                        opt[:], opT[i * 128:(i + 1) * 128,
                                    half * 2048:(half + 1) * 2048])
                    opts[i] = opt
                    for ot in range(4):
                        nc.tensor.matmul(
                            ps[ot][:],
                            avc[0][:, i * sl:(i + 1) * sl],
                            opt[:, ot * 512:(ot + 1) * 512],
                            start=(i == 0), stop=(i == NJ - 1))
                j = i - W
                if j >= 0:
                    opt = opts.pop(j)
                    for ot in range(4):
                        nc.tensor.matmul(
                            ps[4 + ot][:],
                            avc[1][:, j * sl:(j + 1) * sl],
                            opt[:, ot * 512:(ot + 1) * 512],
                            start=(j == 0), stop=(j == NJ - 1))
            for tb in range(2):
                for ot in range(4):
                    oe = oevp.tile([128, 512], F32, tag="oe")
                    if (tb * 4 + ot) % 2 == 0:
                        nc.vector.tensor_copy(oe[:], ps[tb * 4 + ot][:])
                    else:
                        nc.scalar.copy(oe[:], ps[tb * 4 + ot][:])
                    nc.sync.dma_start(
                        out[tb * 128:(tb + 1) * 128,
                            half * 2048 + ot * 512:
                            half * 2048 + (ot + 1) * 512], oe[:])


def prepare_inputs(positions, hidden_states, W_pack, o_proj):
    hs = np.asarray(hidden_states, np.float32).reshape(TQ, HID)
    xT_np = np.ascontiguousarray(hs.T).astype(np.float16)
    opT_np = np.ascontiguousarray(np.asarray(o_proj, np.float32).T
                                  ).astype(np.float16)

    pos = np.asarray(positions, np.int32).reshape(TQ).astype(np.float32)
    inv = (1.0 / THETA ** (np.arange(HD // 2, dtype=np.float32) /
                           (HD // 2))).astype(np.float32)
    ang = inv[:, None] * pos[None, :]              # [64, 2048]
    cos_np = np.concatenate([np.cos(ang), np.cos(ang)], 0).astype(np.float32)
    sin_np = np.concatenate([-np.sin(ang), np.sin(ang)], 0).astype(np.float32)

    kk = np.arange(128)[:, None]
    qq = np.arange(512)[None, :]
    mask_np = np.concatenate(
        [(kk + 128 * dd <= qq).astype(np.float16) for dd in range(4)],
        axis=1)                                     # [128, 2048]
    ones_np = np.ones((128, 8), np.float16)

    Wp = np.asarray(W_pack, np.float32)
    in_maps = []
    for c in range(NCORES):
        r0 = c * JC
        Wc = np.concatenate([Wp[r0:r0 + JC],
                             Wp[HID + r0:HID + r0 + JC],
                             Wp[2 * HID + r0:2 * HID + r0 + JC]], axis=0)
        in_maps.append({
            "xT": xT_np,
            "wT": np.ascontiguousarray(Wc.T).astype(np.float16),
            "opT": opT_np,
            "cosT": cos_np,
            "sinT": sin_np,
            "masks": mask_np,
            "onesI": ones_np,
        })
    return in_maps


_NC_CACHE = None


def _get_program():
    global _NC_CACHE
    if _NC_CACHE is None:
        _NC_CACHE = build_program()
    return _NC_CACHE


def kernel(positions, hidden_states, W_pack, o_proj):
    from concourse.bass_utils import run_bass_kernel_spmd
    nc = _get_program()
    in_maps = prepare_inputs(positions, hidden_states, W_pack, o_proj)
    res = run_bass_kernel_spmd(nc, in_maps, list(range(NCORES)))
    return gather_outputs([res.results[c]["out"] for c in range(NCORES)])


def gather_outputs(outs):
    """Assemble per-core [2*(S/8), HID] slices (rows = batch0 tokens
    c*128..+128 then batch1 same range) into the full [B, S, HID] output."""
    full = np.empty((B, S, HID), np.float32)
    sl = S // NCORES
    for c in range(NCORES):
        o = np.asarray(outs[c]).reshape(B * sl, HID)
        for b in range(B):
            full[b, c * sl:(c + 1) * sl] = o[b * sl:(b + 1) * sl]
    return full


# revision 26
# speedup vs baseline: 1.0289x; 1.0289x over previous
"""Trainium2 Bass kernel for Baichuan attention (B=2, S=1024, HID=4096, NH=32).

Sharding: tensor-parallel over heads (4 heads/core on 8 cores) for
QKV projection + rotary + causal attention; an AllToAll then redistributes
the attention output so every core holds all 4096 features for its own
256-token slice and computes those rows of the final o_proj output with the
full o_proj weight. Host-side gather is a pure concatenation.

On-chip layout is feature-major [feature, token]. QKV/score matmuls run in
float32r (TF32-class mantissa); the softmax weights, V, attention output and
o_proj weight use fp16, whose 10-bit mantissa matches float32r's, halving
their DMA/SBUF footprint at no accuracy cost.
"""
import numpy as np

import concourse.bass as bass
import concourse.mybir as mybir
import concourse.bacc as bacc
import concourse.tile as tile

NCORES = 8
B, S, HID, NH, HD = 2, 1024, 4096, 32, 128
HPC = NH // NCORES          # heads per core = 4
TQ = B * S                  # 2048 tokens
TSL = TQ // NCORES          # 256-token output slice per core
JC = HPC * HD               # 512 features per core per q/k/v
THETA = 10000.0

F32 = mybir.dt.float32
F32R = mybir.dt.float32r
F16 = mybir.dt.float16
AF = mybir.ActivationFunctionType
SCALE = float(HD) ** -0.5


def build_program():
    nc = bacc.Bacc("TRN2", target_bir_lowering=False, debug=False,
                   num_devices=NCORES)
    xT = nc.dram_tensor("xT", [HID, TQ], F16, kind="ExternalInput").ap()
    wT = nc.dram_tensor("wT", [HID, 3 * JC], F16, kind="ExternalInput").ap()
    opT = nc.dram_tensor("opT", [HID, HID], F16, kind="ExternalInput").ap()
    cosT = nc.dram_tensor("cosT", [128, TQ], F32R, kind="ExternalInput").ap()
    sinT = nc.dram_tensor("sinT", [128, TQ], F32R, kind="ExternalInput").ap()
    masks = nc.dram_tensor("masks", [128, 4 * 512], F32R,
                           kind="ExternalInput").ap()
    onesI = nc.dram_tensor("onesI", [128, 8], F32R, kind="ExternalInput").ap()
    out = nc.dram_tensor("out", [TSL, HID], F32, kind="ExternalOutput").ap()

    with tile.TileContext(nc) as tc:
        with tc.tile_pool(name="const", bufs=1) as cp, \
             tc.tile_pool(name="dramp", bufs=1, space="DRAM") as dramp:
            cos_sb = cp.tile([128, TQ], F32R)
            sin_sb = cp.tile([128, TQ], F32R)
            mask_sb = cp.tile([128, 4 * 512], F32R)
            ones_sb = cp.tile([128, 8], F32R)
            nc.scalar.dma_start(cos_sb[:], cosT)
            nc.scalar.dma_start(sin_sb[:], sinT)
            nc.scalar.dma_start(mask_sb[:], masks)
            nc.scalar.dma_start(ones_sb[:], onesI)

            # One AllToAll per batch: batch 0's exchange hides entirely
            # under batch 1's QKV. Each core ends up owning 128 tokens of
            # each batch (s in [c*128, (c+1)*128)).
            a2a_in = [dramp.tile([NCORES, JC, S // NCORES], F16,
                                 name=f"a2a_in{b}") for b in range(B)]
            a2a_out = [dramp.tile([NCORES, JC, S // NCORES], F16,
                                  name=f"a2a_out{b}") for b in range(B)]
            qT_dram = dramp.tile([B, JC, S], F16)

            # avall/opstr opened early so o_proj weight streaming can be
            # prefetched during attention; xslab pool shares per-d tags so
            # batch 1's activation DMAs overlap batch 0's attention.
            with tc.tile_pool(name="avall", bufs=1) as avp, \
                 tc.tile_pool(name="opstr", bufs=13) as opp, \
                 tc.tile_pool(name="psum", bufs=1, space="PSUM") as pspool, \
                 tc.tile_pool(name="xslab", bufs=1) as xp:
                for b in range(B):
                    with tc.tile_pool(name=f"qkv{b}", bufs=1) as qkvp:
                        kT = [qkvp.tile([128, S], F16, name=f"kT{b}_{h}")
                              for h in range(HPC)]
                        vv = [qkvp.tile([128, JC], F32R, name=f"v{b}_{t}")
                              for t in range(8)]
                        _qkv_phase(nc, tc, b, xp, pspool, xT, wT, cos_sb,
                                   sin_sb, kT, vv, qT_dram)
                        _attn_phase(nc, tc, b, pspool, kT, vv, qT_dram,
                                    mask_sb, ones_sb, a2a_in[b])
                    nc.gpsimd.collective_compute(
                        "AllToAll", mybir.AluOpType.bypass,
                        replica_groups=[list(range(NCORES))],
                        ins=[a2a_in[b][:]], outs=[a2a_out[b][:]])

                _oproj_phase(nc, tc, pspool, a2a_out, opT, out, avp, opp)
    nc.compile()
    return nc


def _qkv_phase(nc, tc, b, xp, pspool, xT, wT, cos_sb, sin_sb, kT, vv,
               qT_dram):
    """QKV projection + RoPE for batch b.

    Q/K come out feature-major ([dh, t]; Q spilled to DRAM, K kept in SBUF),
    V token-major ([t, jv]) to serve directly as the AV stationary operand.
    """
    ND = HID // 128  # 32 contraction tiles
    with tc.tile_pool(name=f"wstr{b}", bufs=12) as wp, \
         tc.tile_pool(name=f"rope{b}", bufs=2) as rp, \
         tc.tile_pool(name=f"qev{b}", bufs=2) as qevp:
        xs = []

        # --- Q (jq=0) and K (jq=1), feature-major ---
        for jq in range(2):
            ps = [pspool.tile([128, 512], F32, name=f"ps{b}_{jq}_{i}",
                              tag=f"bk{i}") for i in range(8)]
            for d in range(ND):
                if jq == 0:
                    # just-in-time activation loads: x tile d arrives right
                    # before its first use instead of in one blocking burst
                    xt = xp.tile([128, S], F16, name=f"x{b}_{d}", tag=f"x{d}")
                    nc.sync.dma_start(xt[:], xT[d * 128:(d + 1) * 128,
                                                b * S:(b + 1) * S])
                    xs.append(xt)
                wt = wp.tile([128, 512], F16, tag="wt")
                nc.sync.dma_start(
                    wt[:], wT[d * 128:(d + 1) * 128, jq * 512:(jq + 1) * 512])
                for j in range(4):
                    for ts in range(2):
                        nc.tensor.matmul(
                            ps[j * 2 + ts][:],
                            wt[:, j * 128:(j + 1) * 128],
                            xs[d][:, ts * 512:(ts + 1) * 512],
                            start=(d == 0), stop=(d == ND - 1))
            # Evict all 8 accumulator banks first (alternating engines) so
            # the next pass's matmuls reclaim PSUM immediately; then do the
            # rotary math from SBUF.
            raws = []
            for j in range(4):
                for ts in range(2):
                    raw = rp.tile([128, 512], F16, tag=f"raw{j * 2 + ts}",
                                  bufs=1)
                    if (j + ts) % 2 == 0:
                        nc.scalar.copy(raw[:], ps[j * 2 + ts][:])
                    else:
                        nc.vector.tensor_copy(raw[:], ps[j * 2 + ts][:])
                    raws.append(raw)
            for j in range(4):
                for ts in range(2):
                    raw = raws[j * 2 + ts]
                    tq0 = b * S + ts * 512
                    csl = cos_sb[:, tq0:tq0 + 512]
                    ssl = sin_sb[:, tq0:tq0 + 512]
                    if jq == 1:
                        dest = kT[j][:, ts * 512:(ts + 1) * 512]
                    else:
                        qe = qevp.tile([128, 512], F16, tag="qe")
                        dest = qe[:]
                    sw = rp.tile([128, 512], F16, tag="sw")
                    for qd in range(4):
                        nc.vector.tensor_copy(
                            sw[qd * 32:(qd + 1) * 32, :],
                            raw[(qd * 32 + 64) % 128:
                                (qd * 32 + 64) % 128 + 32, :])
                    nc.vector.tensor_mul(dest, raw[:], csl)
                    nc.vector.tensor_mul(sw[:], sw[:], ssl)
                    nc.vector.tensor_add(dest, dest, sw[:])
                    if jq == 0:
                        nc.scalar.dma_start(
                            qT_dram[b, j * 128:(j + 1) * 128,
                                    ts * 512:(ts + 1) * 512], dest)

        # --- V (jq=2), token-major: psum[t-block] = x_tile.T @ w_v ---
        psv = [pspool.tile([128, 512], F32, name=f"psv{b}_{i}", tag=f"bk{i}")
               for i in range(8)]
        for d in range(ND):
            wt = wp.tile([128, 512], F16, tag="wt")
            nc.sync.dma_start(
                wt[:], wT[d * 128:(d + 1) * 128, 1024:1536])
            for t8 in range(8):
                nc.tensor.matmul(
                    psv[t8][:],
                    xs[d][:, t8 * 128:(t8 + 1) * 128],
                    wt[:],
                    start=(d == 0), stop=(d == ND - 1))
        for t8 in range(8):
            if t8 % 2 == 0:
                nc.scalar.copy(vv[t8][:], psv[t8][:])
            else:
                nc.vector.tensor_copy(vv[t8][:], psv[t8][:])


def _attn_phase(nc, tc, b, pspool, kT, vv, qT_dram, mask_sb, ones_sb,
                a2a_in):
    """Causal attention for batch b: softmax(Q K^T / sqrt(d)) V, 4 heads.

    Works on S^T = K Q^T tiles [k:128, q:512] so the contraction dim (dh,
    then k) always sits on partitions; softmax denominator via a ones-column
    matmul; no max-subtraction (scores are O(10), exp cannot overflow, and
    fp16 underflow of ~e^-18 tail weights is negligible). The score matmul
    runs two k-blocks ahead of the AV/denominator matmuls so the PE never
    waits on the exp/mask latency.
    """
    with tc.tile_pool(name=f"at{b}", bufs=1) as ap:
        cnt = [0, 0]
        for h in range(HPC):
            for qt in range(2):
                qtile = ap.tile([128, 512], F16, tag="qs", bufs=3)
                nc.scalar.dma_start(
                    qtile[:], qT_dram[b, h * 128:(h + 1) * 128,
                                      qt * 512:(qt + 1) * 512])
                psav = pspool.tile([128, 512], F32,
                                   name=f"psav{b}_{h}_{qt}",
                                   tag=f"bk{5 + cnt[1] % 2}")
                psds = pspool.tile([1, 512], F32, name=f"psds{b}_{h}_{qt}",
                                   tag="bk7")
                cnt[1] += 1
                nkb = 4 * (qt + 1)

                def score_tile(kb):
                    pss = pspool.tile([128, 512], F32,
                                      name=f"pss{b}_{h}_{qt}_{kb}",
                                      tag=f"bk{cnt[0] % 5}")
                    cnt[0] += 1
                    nc.tensor.matmul(
                        pss[:], kT[h][:, kb * 128:(kb + 1) * 128], qtile[:],
                        start=True, stop=True)
                    es = ap.tile([128, 512], F32R, tag="es", bufs=6)
                    nc.scalar.activation(es[:], pss[:], AF.Exp, scale=SCALE)
                    dd = kb - 4 * qt
                    if 0 <= dd < 4:
                        nc.vector.tensor_mul(
                            es[:], es[:], mask_sb[:, dd * 512:(dd + 1) * 512])
                    return es

                es_q = [score_tile(k) for k in range(min(4, nkb))]
                for kb in range(nkb):
                    if kb + 4 < nkb:
                        es_q.append(score_tile(kb + 4))
                    es = es_q.pop(0)
                    nc.tensor.matmul(
                        psav[:], vv[kb][:, h * 128:(h + 1) * 128], es[:],
                        start=(kb == 0), stop=(kb == nkb - 1))
                    nc.tensor.matmul(
                        psds[:], ones_sb[:, 0:1], es[:],
                        start=(kb == 0), stop=(kb == nkb - 1))
                recip = ap.tile([1, 512], F32, tag="recip", bufs=2)
                nc.vector.reciprocal_approx_fast(recip[:], psds[:])
                rbc = ap.tile([128, 512], F32, tag="rbc", bufs=2)
                nc.gpsimd.partition_broadcast(rbc[:], recip[:])
                avt = ap.tile([128, 512], F16, tag="avt", bufs=4)
                nc.vector.tensor_mul(avt[:], psav[:], rbc[:])
                for qr in range(4):
                    peer = qt * 4 + qr
                    nc.sync.dma_start(
                        a2a_in[peer, h * 128:(h + 1) * 128, :],
                        avt[:, qr * 128:(qr + 1) * 128])


def _oproj_phase(nc, tc, pspool, a2a_out, opT, out, avp, opp):
    """out rows = [batch0 tokens c*128..+128, batch1 same range] @ o_proj.T."""
    NJ = HID // 128  # 32
    with tc.tile_pool(name="oev", bufs=3) as oevp:
        sl = S // NCORES
        avc = []
        for bb in range(B):
            flat = a2a_out[bb].rearrange("a b c -> (a b) c")
            t = avp.tile([128, NJ * sl], F16, name=f"avc{bb}")
            src = flat.rearrange("(a p) t -> p a t", p=128)
            dst = t[:].rearrange("p (a t) -> p a t", a=NJ)
            for ch in range(8):
                nc.scalar.dma_start(dst[:, ch * 4:(ch + 1) * 4, :],
                                    src[:, ch * 4:(ch + 1) * 4, :])
            avc.append(t)
        # batch-1 (tb=1) matmuls run W iterations behind batch-0's so the
        # final AllToAll's ~30us latency hides under batch-0-only matmuls
        W = 12
        for half in range(2):
            ps = [pspool.tile([128, 512], F32, name=f"pso{half}_{i}",
                              tag=f"bk{i}") for i in range(8)]
            opts = {}
            for i in range(NJ + W):
                if i < NJ:
                    opt = opp.tile([128, 2048], F16, tag="op")
                    nc.scalar.dma_start(
                        opt[:], opT[i * 128:(i + 1) * 128,
                                    half * 2048:(half + 1) * 2048])
                    opts[i] = opt
                    for ot in range(4):
                        nc.tensor.matmul(
                            ps[ot][:],
                            avc[0][:, i * sl:(i + 1) * sl],
                            opt[:, ot * 512:(ot + 1) * 512],
                            start=(i == 0), stop=(i == NJ - 1))
                j = i - W
                if j >= 0:
                    opt = opts.pop(j)
                    for ot in range(4):
                        nc.tensor.matmul(
                            ps[4 + ot][:],
                            avc[1][:, j * sl:(j + 1) * sl],
                            opt[:, ot * 512:(ot + 1) * 512],
                            start=(j == 0), stop=(j == NJ - 1))
            for tb in range(2):
                for ot in range(4):
                    oe = oevp.tile([128, 512], F32, tag="oe")
                    if (tb * 4 + ot) % 2 == 0:
                        nc.vector.tensor_copy(oe[:], ps[tb * 4 + ot][:])
                    else:
                        nc.scalar.copy(oe[:], ps[tb * 4 + ot][:])
                    nc.sync.dma_start(
                        out[tb * 128:(tb + 1) * 128,
                            half * 2048 + ot * 512:
                            half * 2048 + (ot + 1) * 512], oe[:])


def prepare_inputs(positions, hidden_states, W_pack, o_proj):
    hs = np.asarray(hidden_states, np.float32).reshape(TQ, HID)
    xT_np = np.ascontiguousarray(hs.T).astype(np.float16)
    opT_np = np.ascontiguousarray(np.asarray(o_proj, np.float32).T
                                  ).astype(np.float16)

    pos = np.asarray(positions, np.int32).reshape(TQ).astype(np.float32)
    inv = (1.0 / THETA ** (np.arange(HD // 2, dtype=np.float32) /
                           (HD // 2))).astype(np.float32)
    ang = inv[:, None] * pos[None, :]              # [64, 2048]
    cos_np = np.concatenate([np.cos(ang), np.cos(ang)], 0).astype(np.float32)
    sin_np = np.concatenate([-np.sin(ang), np.sin(ang)], 0).astype(np.float32)

    kk = np.arange(128)[:, None]
    qq = np.arange(512)[None, :]
    mask_np = np.concatenate(
        [(kk + 128 * dd <= qq).astype(np.float32) for dd in range(4)],
        axis=1)                                     # [128, 2048]
    ones_np = np.ones((128, 8), np.float32)

    Wp = np.asarray(W_pack, np.float32)
    in_maps = []
    for c in range(NCORES):
        r0 = c * JC
        Wc = np.concatenate([Wp[r0:r0 + JC],
                             Wp[HID + r0:HID + r0 + JC],
                             Wp[2 * HID + r0:2 * HID + r0 + JC]], axis=0)
        in_maps.append({
            "xT": xT_np,
            "wT": np.ascontiguousarray(Wc.T).astype(np.float16),
            "opT": opT_np,
            "cosT": cos_np,
            "sinT": sin_np,
            "masks": mask_np,
            "onesI": ones_np,
        })
    return in_maps


_NC_CACHE = None


def _get_program():
    global _NC_CACHE
    if _NC_CACHE is None:
        _NC_CACHE = build_program()
    return _NC_CACHE


def kernel(positions, hidden_states, W_pack, o_proj):
    from concourse.bass_utils import run_bass_kernel_spmd
    nc = _get_program()
    in_maps = prepare_inputs(positions, hidden_states, W_pack, o_proj)
    res = run_bass_kernel_spmd(nc, in_maps, list(range(NCORES)))
    return gather_outputs([res.results[c]["out"] for c in range(NCORES)])


def gather_outputs(outs):
    """Assemble per-core [2*(S/8), HID] slices (rows = batch0 tokens
    c*128..+128 then batch1 same range) into the full [B, S, HID] output."""
    full = np.empty((B, S, HID), np.float32)
    sl = S // NCORES
    for c in range(NCORES):
        o = np.asarray(outs[c]).reshape(B * sl, HID)
        for b in range(B):
            full[b, c * sl:(c + 1) * sl] = o[b * sl:(b + 1) * sl]
    return full
